# revision 10
# baseline (speedup 1.0000x reference)
"""Trainium2 Bass kernel for nn_AblationAttention (GQA causal attention with
QK-RMSNorm), sharded over 8 NeuronCores.

Problem (hardcoded): B=2, S=2048, E=2048, H=16, KV=8, D=128, G=2.
  q = x@Wq.T+bq; k = x@Wk.T+bk; v = x@Wv.T+bv   (heads split on E)
  q,k: per-head RMSNorm over D (eps = f32 eps), weights qn_w/kn_w
  GQA: kv head c serves q heads 2c, 2c+1; causal softmax(q k / sqrt(D)) @ v
  out = attn_out @ Wo.T + bo

Sharding: core c owns kv head c and q heads {2c, 2c+1} (tensor parallel).
Each core computes its 2 heads' attention output (B,S,256) and a row-parallel
partial of the output projection (B,S,E); host sums the 8 partials (+bo).

On-device layout is "transposed" (feature on partitions, tokens on free dim):
  qT/kT: (D, tokens); scoresT tile: (kt, qt) = kT_chunk.T @ qT  (PE matmul)
  softmax denominators via ones-vector matmuls (partition reduction on PE)
  out_T = v_chunk.T @ expT accumulated over kt chunks
Matmuls run in float32r (1 cycle/row at free-dim>=256, ~1e-3 rel err).
"""

import os
import sys

for _p in ("/opt/trn_rl_repo", "/root/.axon_site/_ro/trn_rl_repo"):
    if os.path.isdir(_p) and _p not in sys.path:
        sys.path.append(_p)

import numpy as np

import concourse.bass as bass
import concourse.tile as tile
from concourse import bacc, mybir
from concourse.bass_utils import run_bass_kernel_spmd
from concourse.masks import make_identity

if os.environ.get("KERNEL_LDW_OPT") == "1":
    from concourse import bass_utils as _bu

    _orig_run_command = _bu.run_command

    def _patched_run_command(argv, **kwargs):
        argv = [
            a.replace("--enable-ldw-opt=false", "--enable-ldw-opt=true")
            if isinstance(a, str)
            else a
            for a in argv
        ]
        return _orig_run_command(argv, **kwargs)

    _bu.run_command = _patched_run_command

B, S, E = 2, 2048, 2048
H, KV, D = 16, 8, 128
G = H // KV
BS = B * S  # 4096 tokens
EPS = float(np.finfo(np.float32).eps)
NCORES = 8

TB = 512  # token block (psum free dim)
NTB = BS // TB  # 8 global token blocks (4 per batch)
NKI = S // 128  # 16 key chunks per batch
NQI = S // TB  # 4 query blocks per batch
NEG = -1.0e30

F32 = mybir.dt.float32
F32R = mybir.dt.float32r


def _r(ap):
    """Matmul operand helper (tiles are already float32r)."""
    return ap


def build_nc():
    nc = bacc.Bacc()

    xT = nc.dram_tensor("xT", [E, BS], F32, kind="ExternalInput")
    wqkvT = nc.dram_tensor("wqkvT", [E, 512], F32, kind="ExternalInput")
    woT = nc.dram_tensor("woT", [2 * D, E], F32, kind="ExternalInput")
    bqkv = nc.dram_tensor("bqkv", [128, 4], F32, kind="ExternalInput")
    wqk = nc.dram_tensor("wqk", [128, 1], F32, kind="ExternalInput")
    masks = nc.dram_tensor("masks", [128, 4, TB], F32, kind="ExternalInput")
    yT = nc.dram_tensor("yT", [E, BS], F32, kind="ExternalOutput")

    with tile.TileContext(nc) as tc:
        with (
            tc.tile_pool(name="consts", bufs=1) as consts,
            tc.tile_pool(name="acts", bufs=1) as acts,
        ):
            # resident weights / constants (staged: f32r matmul operands must
            # be produced by a rounding engine op, not raw DMA)
            w_sb = consts.tile([128, E // 128, 512], F32R)
            wo_sb = consts.tile([128, 2, E], F32R)
            b_sb = consts.tile([128, 4], F32)
            nc.sync.dma_start(out=b_sb, in_=bqkv[:, :])
            wqk_sb = consts.tile([128, 1], F32)
            nc.sync.dma_start(out=wqk_sb, in_=wqk[:, :])
            mask_sb = consts.tile([128, 4, TB], F32)
            nc.sync.dma_start(out=mask_sb, in_=masks[:, :, :])
            ident = consts.tile([128, 128], F32)
            make_identity(nc, ident)
            ones_f32 = consts.tile([128, 128], F32)
            nc.vector.memset(ones_f32, 1.0)
            ones_red = consts.tile([128, 1], F32R)
            nc.vector.tensor_copy(ones_red, ones_f32[:, 0:1])
            ones_row = consts.tile([1, 128], F32R)
            nc.vector.tensor_copy(ones_row, ones_f32[0:1, :])
            eps_sb = consts.tile([128, 1], F32)
            nc.vector.memset(eps_sb, EPS)
            deps_sb = consts.tile([128, 1], F32)
            nc.vector.memset(deps_sb, float(D) * EPS)

            # resident activations
            q_sb = acts.tile([128, G, BS], F32R)  # qT per local head
            k_sb = acts.tile([128, BS], F32R)  # kT
            vn_sb = acts.tile([128, NTB * 4, 128], F32R)  # v natural (tok, d)
            beta_sb = acts.tile([128, B, NKI], F32)  # combined k-rms * 1/sqrt(D)
            alpha_sb = acts.tile([1, S], F32R)  # q-rms factors (per b,h reuse)
            sqms_sb = acts.tile([1, S], F32)

            # ---------------- Phase 1: QKV projection + v transpose ----------
            with (
                tc.tile_pool(name="xp", bufs=6) as xp,
                tc.tile_pool(name="vtp", bufs=2) as vtp,
                tc.tile_pool(name="ps1", bufs=1, space="PSUM") as ps1,
                tc.tile_pool(name="psvt", bufs=2, space="PSUM") as psvt,
            ):
                for tb in range(NTB):
                    ps_acc = [
                        ps1.tile([128, TB], F32, tag=f"acc{j}", name=f"acc{j}")
                        for j in range(4)
                    ]
                    for et in range(E // 128):
                        if tb == 0:
                            wst = xp.tile([128, 512], F32, tag="x", name="wst")
                            nc.sync.dma_start(
                                out=wst, in_=wqkvT[et * 128 : (et + 1) * 128, :]
                            )
                            if et % 2 == 0:
                                nc.vector.tensor_copy(w_sb[:, et, :], wst)
                            else:
                                nc.scalar.copy(out=w_sb[:, et, :], in_=wst)
                        x_s = xp.tile([128, TB], F32, tag="x", name="x_s")
                        nc.sync.dma_start(
                            out=x_s,
                            in_=xT[et * 128 : (et + 1) * 128, tb * TB : (tb + 1) * TB],
                        )
                        x_t = xp.tile([128, TB], F32R, tag="xr", name="x_t")
                        if et % 2 == 0:
                            nc.vector.tensor_copy(x_t, x_s)
                        else:
                            nc.scalar.copy(out=x_t, in_=x_s)
                        for j in range(4):
                            nc.tensor.matmul(
                                ps_acc[j],
                                _r(w_sb[:, et, j * 128 : (j + 1) * 128]),
                                _r(x_t),
                                start=(et == 0),
                                stop=(et == E // 128 - 1),
                            )
                    # psum -> sbuf with bias add (ACT, per-partition bias)
                    for j in range(2):  # q heads
                        nc.scalar.activation(
                            out=q_sb[:, j, tb * TB : (tb + 1) * TB],
                            in_=ps_acc[j],
                            func=mybir.ActivationFunctionType.Identity,
                            bias=b_sb[:, j : j + 1],
                            scale=1.0,
                        )
                    nc.vector.tensor_scalar(
                        out=k_sb[:, tb * TB : (tb + 1) * TB],
                        in0=ps_acc[2],
                        scalar1=b_sb[:, 2:3],
                        scalar2=None,
                        op0=mybir.AluOpType.add,
                    )
                    vt_tmp = vtp.tile([128, TB], F32, tag="vt")
                    nc.vector.tensor_scalar(
                        out=vt_tmp,
                        in0=ps_acc[3],
                        scalar1=b_sb[:, 3:4],
                        scalar2=None,
                        op0=mybir.AluOpType.add,
                    )
                    for t in range(4):
                        vt_ps = psvt.tile([128, 128], F32, tag="vtps")
                        nc.tensor.transpose(
                            vt_ps, vt_tmp[:, t * 128 : (t + 1) * 128], ident
                        )
                        nc.vector.tensor_copy(vn_sb[:, tb * 4 + t, :], vt_ps)
                for ct in range(2):
                    for es in range(E // 512):
                        wst = xp.tile([128, 512], F32, tag="x", name="wst")
                        nc.sync.dma_start(
                            out=wst,
                            in_=woT[
                                ct * 128 : (ct + 1) * 128, es * 512 : (es + 1) * 512
                            ],
                        )
                        if es % 2 == 0:
                            nc.vector.tensor_copy(
                                wo_sb[:, ct, es * 512 : (es + 1) * 512], wst
                            )
                        else:
                            nc.scalar.copy(
                                out=wo_sb[:, ct, es * 512 : (es + 1) * 512], in_=wst
                            )

            # ---------------- Phase 2: RMS factors (Sqrt table set) ----------
            with (
                tc.tile_pool(name="sqp", bufs=3) as sqp,
                tc.tile_pool(name="ps2", bufs=2, space="PSUM") as ps2,
                tc.tile_pool(name="ps2b", bufs=2, space="PSUM") as ps2b,
            ):
                # K: per-token 1/(sqrt(D)*sqrt(ms+eps)) in (kt, chunk) layout
                for b in range(B):
                    for ki in range(NKI):
                        sl = slice(b * S + ki * 128, b * S + (ki + 1) * 128)
                        sqk = sqp.tile([128, 128], F32, tag="sqk")
                        nc.vector.tensor_mul(sqk, k_sb[:, sl], k_sb[:, sl])
                        bt_ps = ps2b.tile([128, 1], F32, tag="btp")
                        nc.tensor.matmul(
                            bt_ps, sqk, ones_f32[:, 0:1], start=True, stop=True
                        )
                        nc.scalar.activation(
                            out=beta_sb[:, b, ki : ki + 1],
                            in_=bt_ps,
                            func=mybir.ActivationFunctionType.Sqrt,
                            bias=deps_sb,
                            scale=1.0,
                        )
                for b in range(B):
                    nc.vector.reciprocal(beta_sb[:, b, :], beta_sb[:, b, :])
                # fold qn_w * kn_w into k (valid: post-RMS per-d scale)
                nc.vector.tensor_scalar_mul(k_sb, in0=k_sb, scalar1=wqk_sb)

                # Q: alpha = 1/sqrt(ms+eps) per token; multiply into qT columns
                for b in range(B):
                    for h in range(G):
                        qsl = q_sb[:, h, b * S : (b + 1) * S]
                        for t in range(NQI):
                            tsl = slice(t * TB, (t + 1) * TB)
                            sq = sqp.tile([128, TB], F32R, tag="sq")
                            nc.vector.tensor_mul(sq, qsl[:, tsl], qsl[:, tsl])
                            sm_ps = ps2.tile([1, TB], F32, tag="smp")
                            nc.tensor.matmul(
                                sm_ps, _r(ones_red), _r(sq), start=True, stop=True
                            )
                            nc.scalar.activation(
                                out=sqms_sb[:, tsl],
                                in_=sm_ps,
                                func=mybir.ActivationFunctionType.Sqrt,
                                bias=eps_sb[:1],
                                scale=1.0 / D,
                            )
                        with nc.allow_low_precision(reason="f32r rounding for PE"):
                            nc.vector.reciprocal(alpha_sb, sqms_sb)
                        for t in range(NQI):
                            tsl = slice(t * TB, (t + 1) * TB)
                            bc_ps = ps2.tile([128, TB], F32, tag="bcp")
                            nc.tensor.matmul(
                                bc_ps,
                                _r(ones_row),
                                _r(alpha_sb[:, tsl]),
                                start=True,
                                stop=True,
                            )
                            nc.vector.tensor_mul(qsl[:, tsl], qsl[:, tsl], bc_ps)

            # ---------------- Phase 3+4: attention + out projection ----------
            with (
                tc.tile_pool(name="expp", bufs=4) as expp,
                tc.tile_pool(name="osh", bufs=2) as osh,
                tc.tile_pool(name="yp", bufs=4) as ypool,
                tc.tile_pool(name="linvp", bufs=2) as linvp,
                tc.tile_pool(name="ps3", bufs=3, space="PSUM") as ps3,
                tc.tile_pool(name="ps3o", bufs=2, space="PSUM") as ps3o,
                tc.tile_pool(name="ps3l", bufs=2, space="PSUM") as ps3l,
                tc.tile_pool(name="ps3b", bufs=1, space="PSUM") as ps3b,
            ):
                for b in range(B):
                    out_b = osh.tile([128, G, S], F32R, tag="outsh")
                    for h in range(G):
                        qsl = q_sb[:, h, b * S : (b + 1) * S]
                        pending_norm = None
                        for qi in range(NQI):
                            nki = 4 * qi + 4
                            qblk = _r(qsl[:, qi * TB : (qi + 1) * TB])
                            o_ps = ps3o.tile([128, TB], F32, tag="op")
                            l_ps = ps3l.tile([1, TB], F32, tag="lp")

                            def scores(ki, sc_ps):
                                ksl = k_sb[
                                    :, b * S + ki * 128 : b * S + (ki + 1) * 128
                                ]
                                nc.tensor.matmul(
                                    sc_ps, _r(ksl), qblk, start=True, stop=True
                                )
                                if ki >= 4 * qi:  # diagonal chunk: causal mask
                                    nc.vector.tensor_add(
                                        sc_ps, sc_ps, mask_sb[:, ki - 4 * qi, :]
                                    )

                            sc_tiles = {}
                            LOOKAHEAD = 2
                            for ki in range(min(LOOKAHEAD, nki)):
                                sc_tiles[ki] = ps3.tile([128, TB], F32, tag="sc", name="sc")
                                scores(ki, sc_tiles[ki])
                            if pending_norm is not None:
                                pending_norm()
                                pending_norm = None
                            for ki in range(nki):
                                if ki + LOOKAHEAD < nki:
                                    sc_tiles[ki + LOOKAHEAD] = ps3.tile(
                                        [128, TB], F32, tag="sc", name="sc"
                                    )
                                    scores(ki + LOOKAHEAD, sc_tiles[ki + LOOKAHEAD])
                                e_sb = expp.tile([128, TB], F32R, tag="exp")
                                nc.scalar.activation(
                                    out=e_sb,
                                    in_=sc_tiles.pop(ki),
                                    func=mybir.ActivationFunctionType.Exp,
                                    scale=beta_sb[:, b, ki : ki + 1],
                                )
                                nc.tensor.matmul(
                                    o_ps,
                                    _r(vn_sb[:, b * NKI + ki, :]),
                                    _r(e_sb),
                                    start=(ki == 0),
                                    stop=(ki == nki - 1),
                                )
                                nc.tensor.matmul(
                                    l_ps,
                                    _r(ones_red),
                                    _r(e_sb),
                                    start=(ki == 0),
                                    stop=(ki == nki - 1),
                                )
                            def make_norm(qi, o_ps, l_ps):
                                def norm():
                                    linv = linvp.tile(
                                        [1, TB], F32R, tag="linv", name="linv"
                                    )
                                    with nc.allow_low_precision(
                                        reason="f32r rounding for PE"
                                    ):
                                        nc.vector.reciprocal(linv, l_ps)
                                    bc_ps = ps3b.tile(
                                        [128, TB], F32, tag="nbc", name="bc_ps"
                                    )
                                    nc.tensor.matmul(
                                        bc_ps,
                                        _r(ones_row),
                                        _r(linv),
                                        start=True,
                                        stop=True,
                                    )
                                    osl = out_b[:, h, qi * TB : (qi + 1) * TB]
                                    nc.scalar.copy(out=osl, in_=o_ps)
                                    nc.vector.tensor_mul(osl, osl, bc_ps)

                                return norm

                            pending_norm = make_norm(qi, o_ps, l_ps)
                        if pending_norm is not None:
                            pending_norm()
                            pending_norm = None

                    # out projection for this batch (row-parallel partial)
                    for t in range(NQI):
                        for et in range(E // 128):
                            y_ps = ps3o.tile([128, TB], F32, tag="op", name="y_ps")
                            for ct in range(2):
                                nc.tensor.matmul(
                                    y_ps,
                                    _r(wo_sb[:, ct, et * 128 : (et + 1) * 128]),
                                    _r(out_b[:, ct, t * TB : (t + 1) * TB]),
                                    start=(ct == 0),
                                    stop=(ct == 1),
                                )
                            y_sb = ypool.tile([128, TB], F32, tag="y")
                            if et % 2 == 0:
                                nc.scalar.copy(out=y_sb, in_=y_ps)
                            else:
                                nc.vector.tensor_copy(y_sb, y_ps)
                            nc.sync.dma_start(
                                out=yT[
                                    et * 128 : (et + 1) * 128,
                                    (b * NQI + t) * TB : (b * NQI + t + 1) * TB,
                                ],
                                in_=y_sb,
                            )
    nc.compile()
    return nc


def _prep_inputs(x, Wq, bq, Wk, bk, Wv, bv, Wo, bo, qn_w, kn_w):
    """Shard the full inputs into the 8 per-core input maps."""
    x = np.asarray(x, np.float32)
    xT = np.ascontiguousarray(x.reshape(BS, E).T)  # (E, BS)

    # causal masks for the 4 diagonal (kt=128, qt=512) tile offsets
    kt = np.arange(128)[:, None]
    qt = np.arange(TB)[None, :]
    masks = np.stack(
        [np.where(qt >= kt + 128 * j, 0.0, NEG).astype(np.float32) for j in range(4)],
        axis=1,
    )  # (128, 4, 512)
    masks = np.ascontiguousarray(masks)

    wqk = np.ascontiguousarray(
        (np.asarray(qn_w, np.float32) * np.asarray(kn_w, np.float32)).reshape(128, 1)
    )

    in_maps = []
    for c in range(NCORES):
        qrows = slice(2 * c * D, (2 * c + 2) * D)
        kvrows = slice(c * D, (c + 1) * D)
        wcat = np.concatenate(
            [np.asarray(Wq, np.float32)[qrows], np.asarray(Wk, np.float32)[kvrows],
             np.asarray(Wv, np.float32)[kvrows]], axis=0
        )  # (512, E)
        wqkvT = np.ascontiguousarray(wcat.T)  # (E, 512)
        woT = np.ascontiguousarray(np.asarray(Wo, np.float32)[:, qrows].T)  # (256, E)
        bcat = np.concatenate(
            [np.asarray(bq, np.float32)[qrows], np.asarray(bk, np.float32)[kvrows],
             np.asarray(bv, np.float32)[kvrows]]
        )  # (512,)
        bqkv = np.ascontiguousarray(bcat.reshape(4, 128).T)  # (128, 4)
        in_maps.append(
            {
                "xT": xT,
                "wqkvT": wqkvT,
                "woT": woT,
                "bqkv": bqkv,
                "wqk": wqk,
                "masks": masks,
            }
        )
    return in_maps


def _unshard(results, bo):
    acc = np.zeros((E, BS), np.float64)
    for r in results:
        acc += r["yT"].astype(np.float64)
    y = acc.T.reshape(B, S, E) + np.asarray(bo, np.float64)[None, None, :]
    return y.astype(np.float32)


_NC_CACHE = []


def _get_nc():
    if not _NC_CACHE:
        _NC_CACHE.append(build_nc())
    return _NC_CACHE[0]


def run(inputs, trace=False):
    nc = _get_nc()
    in_maps = _prep_inputs(**inputs)
    res = run_bass_kernel_spmd(
        nc, in_maps, core_ids=list(range(NCORES)), trace=trace
    )
    out = _unshard(res.results, inputs["bo"])
    return out, res


def kernel(**inputs) -> np.ndarray:
    out, _ = run(inputs, trace=False)
    return out


# revision 11
# speedup vs baseline: 1.3941x; 1.3941x over previous
"""Trainium2 Bass kernel for nn_AblationAttention (GQA causal attention with
QK-RMSNorm), sharded over 8 NeuronCores.

Problem (hardcoded): B=2, S=2048, E=2048, H=16, KV=8, D=128, G=2.
  q = x@Wq.T+bq; k = x@Wk.T+bk; v = x@Wv.T+bv   (heads split on E)
  q,k: per-head RMSNorm over D (eps = f32 eps), weights qn_w/kn_w
  GQA: kv head c serves q heads 2c, 2c+1; causal softmax(q k / sqrt(D)) @ v
  out = attn_out @ Wo.T + bo

Sharding: core c owns kv head c and q heads {2c, 2c+1} (tensor parallel).
Each core computes its 2 heads' attention output (B,S,256) and a row-parallel
partial of the output projection (B,S,E); host sums the 8 partials (+bo).

On-device layout is "transposed" (feature on partitions, tokens on free dim):
  qT/kT: (D, tokens); scoresT tile: (kt, qt) = kT_chunk.T @ qT  (PE matmul)
  softmax denominators via ones-vector matmuls (partition reduction on PE)
  out_T = v_chunk.T @ expT accumulated over kt chunks
Matmuls run in float32r (1 cycle/row at free-dim>=256, ~1e-3 rel err).
"""

import os
import sys

for _p in ("/opt/trn_rl_repo", "/root/.axon_site/_ro/trn_rl_repo"):
    if os.path.isdir(_p) and _p not in sys.path:
        sys.path.append(_p)

import numpy as np

import concourse.bass as bass
import concourse.tile as tile
from concourse import bacc, mybir
from concourse.bass_utils import run_bass_kernel_spmd
from concourse.masks import make_identity

if os.environ.get("KERNEL_LDW_OPT") == "1":
    from concourse import bass_utils as _bu

    _orig_run_command = _bu.run_command

    def _patched_run_command(argv, **kwargs):
        argv = [
            a.replace("--enable-ldw-opt=false", "--enable-ldw-opt=true")
            if isinstance(a, str)
            else a
            for a in argv
        ]
        return _orig_run_command(argv, **kwargs)

    _bu.run_command = _patched_run_command

B, S, E = 2, 2048, 2048
H, KV, D = 16, 8, 128
G = H // KV
BS = B * S  # 4096 tokens
EPS = float(np.finfo(np.float32).eps)
NCORES = 8

TB = 512  # token block (psum free dim)
NTB = BS // TB  # 8 global token blocks (4 per batch)
NKI = S // 128  # 16 key chunks per batch
NQI = S // TB  # 4 query blocks per batch
NEG = -1.0e30

F32 = mybir.dt.float32
F32R = mybir.dt.float32r


def _r(ap):
    """Matmul operand helper (tiles are already float32r)."""
    return ap


def build_nc():
    nc = bacc.Bacc()

    xT = nc.dram_tensor("xT", [E, BS], F32, kind="ExternalInput")
    wqkvT = nc.dram_tensor("wqkvT", [E, 512], F32, kind="ExternalInput")
    woT = nc.dram_tensor("woT", [2 * D, E], F32, kind="ExternalInput")
    bqkv = nc.dram_tensor("bqkv", [128, 4], F32, kind="ExternalInput")
    wqk = nc.dram_tensor("wqk", [128, 1], F32, kind="ExternalInput")
    masks = nc.dram_tensor("masks", [128, 4, TB], F32, kind="ExternalInput")
    yT = nc.dram_tensor("yT", [E, BS], F32, kind="ExternalOutput")

    with tile.TileContext(nc) as tc:
        with (
            tc.tile_pool(name="consts", bufs=1) as consts,
            tc.tile_pool(name="acts", bufs=1) as acts,
        ):
            # resident weights / constants (staged: f32r matmul operands must
            # be produced by a rounding engine op, not raw DMA)
            w_sb = consts.tile([128, E // 128, 512], F32R)
            wo_sb = consts.tile([128, 2, E], F32R)
            b_sb = consts.tile([128, 4], F32)
            nc.sync.dma_start(out=b_sb, in_=bqkv[:, :])
            wqk_sb = consts.tile([128, 1], F32)
            nc.sync.dma_start(out=wqk_sb, in_=wqk[:, :])
            mask_sb = consts.tile([128, 4, TB], F32)
            nc.sync.dma_start(out=mask_sb, in_=masks[:, :, :])
            ident = consts.tile([128, 128], F32)
            make_identity(nc, ident)
            ones_f32 = consts.tile([128, 128], F32)
            nc.vector.memset(ones_f32, 1.0)
            ones_red = consts.tile([128, 1], F32R)
            nc.vector.tensor_copy(ones_red, ones_f32[:, 0:1])
            ones_sq = consts.tile([128, 128], F32R)
            nc.vector.tensor_copy(ones_sq, ones_f32)
            ones_row = consts.tile([1, 128], F32R)
            nc.vector.tensor_copy(ones_row, ones_f32[0:1, :])
            eps_sb = consts.tile([128, 1], F32)
            nc.vector.memset(eps_sb, EPS)
            deps_sb = consts.tile([128, 1], F32)
            nc.vector.memset(deps_sb, float(D) * EPS)

            # resident activations
            q_sb = acts.tile([128, G, BS], F32R)  # qT per local head
            k_sb = acts.tile([128, BS], F32R)  # kT
            vn_sb = acts.tile([128, NTB * 4, 128], F32R)  # v natural (tok, d)
            beta_sb = acts.tile([128, B, NKI], F32)  # combined k-rms * 1/sqrt(D)
            alpha_sb = acts.tile([1, S], F32R)  # q-rms factors (per b,h reuse)
            sqms_sb = acts.tile([1, S], F32)

            # ---------------- Phase 1: QKV projection + v transpose ----------
            with (
                tc.tile_pool(name="xp", bufs=8) as xp,
                tc.tile_pool(name="vtp", bufs=2) as vtp,
                tc.tile_pool(name="ps1", bufs=1, space="PSUM") as ps1,
                tc.tile_pool(name="psvt", bufs=2, space="PSUM") as psvt,
            ):
                for tb in range(NTB):
                    ps_acc = [
                        ps1.tile([128, TB], F32, tag=f"acc{j}", name=f"acc{j}")
                        for j in range(4)
                    ]
                    for et in range(E // 128):
                        if tb == 0:
                            wst = xp.tile([128, 512], F32, tag="x", name="wst")
                            nc.sync.dma_start(
                                out=wst, in_=wqkvT[et * 128 : (et + 1) * 128, :]
                            )
                            if et % 2 == 0:
                                nc.vector.tensor_copy(w_sb[:, et, :], wst)
                            else:
                                nc.scalar.copy(out=w_sb[:, et, :], in_=wst)
                        x_s = xp.tile([128, TB], F32, tag="x", name="x_s")
                        nc.sync.dma_start(
                            out=x_s,
                            in_=xT[et * 128 : (et + 1) * 128, tb * TB : (tb + 1) * TB],
                        )
                        x_t = xp.tile([128, TB], F32R, tag="xr", name="x_t")
                        if et % 2 == 0:
                            nc.vector.tensor_copy(x_t, x_s)
                        else:
                            nc.scalar.copy(out=x_t, in_=x_s)
                        for j in range(4):
                            nc.tensor.matmul(
                                ps_acc[j],
                                _r(w_sb[:, et, j * 128 : (j + 1) * 128]),
                                _r(x_t),
                                start=(et == 0),
                                stop=(et == E // 128 - 1),
                            )
                    # psum -> sbuf with bias add (ACT, per-partition bias)
                    for j in range(2):  # q heads
                        nc.scalar.activation(
                            out=q_sb[:, j, tb * TB : (tb + 1) * TB],
                            in_=ps_acc[j],
                            func=mybir.ActivationFunctionType.Identity,
                            bias=b_sb[:, j : j + 1],
                            scale=1.0,
                        )
                    nc.vector.tensor_scalar(
                        out=k_sb[:, tb * TB : (tb + 1) * TB],
                        in0=ps_acc[2],
                        scalar1=b_sb[:, 2:3],
                        scalar2=None,
                        op0=mybir.AluOpType.add,
                    )
                    vt_tmp = vtp.tile([128, TB], F32, tag="vt")
                    nc.vector.tensor_scalar(
                        out=vt_tmp,
                        in0=ps_acc[3],
                        scalar1=b_sb[:, 3:4],
                        scalar2=None,
                        op0=mybir.AluOpType.add,
                    )
                    for t in range(4):
                        vt_ps = psvt.tile([128, 128], F32, tag="vtps")
                        nc.tensor.transpose(
                            vt_ps, vt_tmp[:, t * 128 : (t + 1) * 128], ident
                        )
                        nc.vector.tensor_copy(vn_sb[:, tb * 4 + t, :], vt_ps)
                for ct in range(2):
                    for es in range(E // 512):
                        wst = xp.tile([128, 512], F32, tag="x", name="wst")
                        nc.sync.dma_start(
                            out=wst,
                            in_=woT[
                                ct * 128 : (ct + 1) * 128, es * 512 : (es + 1) * 512
                            ],
                        )
                        if es % 2 == 0:
                            nc.vector.tensor_copy(
                                wo_sb[:, ct, es * 512 : (es + 1) * 512], wst
                            )
                        else:
                            nc.scalar.copy(
                                out=wo_sb[:, ct, es * 512 : (es + 1) * 512], in_=wst
                            )

            # ---------------- Phase 2: RMS factors (Sqrt table set) ----------
            with (
                tc.tile_pool(name="sqp", bufs=3) as sqp,
                tc.tile_pool(name="ps2", bufs=2, space="PSUM") as ps2,
                tc.tile_pool(name="ps2b", bufs=2, space="PSUM") as ps2b,
            ):
                # K: per-token 1/(sqrt(D)*sqrt(ms+eps)) in (kt, chunk) layout
                for b in range(B):
                    for ki in range(NKI):
                        sl = slice(b * S + ki * 128, b * S + (ki + 1) * 128)
                        sqk = sqp.tile([128, 128], F32, tag="sqk")
                        nc.vector.tensor_mul(sqk, k_sb[:, sl], k_sb[:, sl])
                        bt_ps = ps2b.tile([128, 1], F32, tag="btp")
                        nc.tensor.matmul(
                            bt_ps, sqk, ones_f32[:, 0:1], start=True, stop=True
                        )
                        nc.scalar.activation(
                            out=beta_sb[:, b, ki : ki + 1],
                            in_=bt_ps,
                            func=mybir.ActivationFunctionType.Sqrt,
                            bias=deps_sb,
                            scale=1.0,
                        )
                for b in range(B):
                    nc.vector.reciprocal(beta_sb[:, b, :], beta_sb[:, b, :])
                # fold qn_w * kn_w into k (valid: post-RMS per-d scale)
                nc.vector.tensor_scalar_mul(k_sb, in0=k_sb, scalar1=wqk_sb)

                # Q: alpha = 1/sqrt(ms+eps) per token; multiply into qT columns
                for b in range(B):
                    for h in range(G):
                        qsl = q_sb[:, h, b * S : (b + 1) * S]
                        for t in range(NQI):
                            tsl = slice(t * TB, (t + 1) * TB)
                            sq = sqp.tile([128, TB], F32R, tag="sq")
                            nc.vector.tensor_mul(sq, qsl[:, tsl], qsl[:, tsl])
                            sm_ps = ps2.tile([1, TB], F32, tag="smp")
                            nc.tensor.matmul(
                                sm_ps, _r(ones_red), _r(sq), start=True, stop=True
                            )
                            nc.scalar.activation(
                                out=sqms_sb[:, tsl],
                                in_=sm_ps,
                                func=mybir.ActivationFunctionType.Sqrt,
                                bias=eps_sb[:1],
                                scale=1.0 / D,
                            )
                        with nc.allow_low_precision(reason="f32r rounding for PE"):
                            nc.vector.reciprocal(alpha_sb, sqms_sb)
                        for t in range(NQI):
                            tsl = slice(t * TB, (t + 1) * TB)
                            bc_ps = ps2.tile([128, TB], F32, tag="bcp")
                            nc.tensor.matmul(
                                bc_ps,
                                _r(ones_row),
                                _r(alpha_sb[:, tsl]),
                                start=True,
                                stop=True,
                            )
                            nc.vector.tensor_mul(qsl[:, tsl], qsl[:, tsl], bc_ps)

            # ---------------- Phase 3+4: attention + out projection ----------
            with (
                tc.tile_pool(name="expp", bufs=4) as expp,
                tc.tile_pool(name="osh", bufs=2) as osh,
                tc.tile_pool(name="yp", bufs=4) as ypool,
                tc.tile_pool(name="linvp", bufs=2) as linvp,
                tc.tile_pool(name="ps3", bufs=4, space="PSUM") as ps3,
                tc.tile_pool(name="ps3o", bufs=2, space="PSUM") as ps3o,
                tc.tile_pool(name="ps3l", bufs=2, space="PSUM") as ps3l,
            ):
                for b in range(B):
                    out_b = osh.tile([128, G, S], F32R, tag="outsh")
                    for h in range(G):
                        qsl = q_sb[:, h, b * S : (b + 1) * S]
                        for qi in range(NQI):
                            nki = 4 * qi + 4
                            qblk = _r(qsl[:, qi * TB : (qi + 1) * TB])
                            o_ps = ps3o.tile([128, TB], F32, tag="op")
                            l_ps = ps3l.tile([128, TB], F32, tag="lp")

                            def scores(ki, sc_ps):
                                ksl = k_sb[
                                    :, b * S + ki * 128 : b * S + (ki + 1) * 128
                                ]
                                nc.tensor.matmul(
                                    sc_ps, _r(ksl), qblk, start=True, stop=True
                                )
                                if ki >= 4 * qi:  # diagonal chunk: causal mask
                                    nc.vector.tensor_add(
                                        sc_ps, sc_ps, mask_sb[:, ki - 4 * qi, :]
                                    )

                            sc_tiles = {}
                            LOOKAHEAD = 3
                            for ki in range(min(LOOKAHEAD, nki)):
                                sc_tiles[ki] = ps3.tile([128, TB], F32, tag="sc", name="sc")
                                scores(ki, sc_tiles[ki])
                            for ki in range(nki):
                                if ki + LOOKAHEAD < nki:
                                    sc_tiles[ki + LOOKAHEAD] = ps3.tile(
                                        [128, TB], F32, tag="sc", name="sc"
                                    )
                                    scores(ki + LOOKAHEAD, sc_tiles[ki + LOOKAHEAD])
                                e_sb = expp.tile([128, TB], F32R, tag="exp")
                                nc.scalar.activation(
                                    out=e_sb,
                                    in_=sc_tiles.pop(ki),
                                    func=mybir.ActivationFunctionType.Exp,
                                    scale=beta_sb[:, b, ki : ki + 1],
                                )
                                nc.tensor.matmul(
                                    o_ps,
                                    _r(vn_sb[:, b * NKI + ki, :]),
                                    _r(e_sb),
                                    start=(ki == 0),
                                    stop=(ki == nki - 1),
                                )
                                nc.tensor.matmul(
                                    l_ps,
                                    _r(ones_sq),
                                    _r(e_sb),
                                    start=(ki == 0),
                                    stop=(ki == nki - 1),
                                )
                            linv = linvp.tile(
                                [128, TB], F32, tag="linv", name="linv"
                            )
                            nc.vector.reciprocal_approx_fast(linv, l_ps)
                            nc.vector.tensor_mul(
                                out_b[:, h, qi * TB : (qi + 1) * TB], linv, o_ps
                            )

                    # out projection for this batch (row-parallel partial)
                    for t in range(NQI):
                        for et in range(E // 128):
                            y_ps = ps3o.tile([128, TB], F32, tag="op", name="y_ps")
                            for ct in range(2):
                                nc.tensor.matmul(
                                    y_ps,
                                    _r(wo_sb[:, ct, et * 128 : (et + 1) * 128]),
                                    _r(out_b[:, ct, t * TB : (t + 1) * TB]),
                                    start=(ct == 0),
                                    stop=(ct == 1),
                                )
                            y_sb = ypool.tile([128, TB], F32, tag="y")
                            if et % 2 == 0:
                                nc.scalar.copy(out=y_sb, in_=y_ps)
                            else:
                                nc.vector.tensor_copy(y_sb, y_ps)
                            nc.sync.dma_start(
                                out=yT[
                                    et * 128 : (et + 1) * 128,
                                    (b * NQI + t) * TB : (b * NQI + t + 1) * TB,
                                ],
                                in_=y_sb,
                            )
    nc.compile()
    return nc


def _prep_inputs(x, Wq, bq, Wk, bk, Wv, bv, Wo, bo, qn_w, kn_w):
    """Shard the full inputs into the 8 per-core input maps."""
    x = np.asarray(x, np.float32)
    xT = np.ascontiguousarray(x.reshape(BS, E).T)  # (E, BS)

    # causal masks for the 4 diagonal (kt=128, qt=512) tile offsets
    kt = np.arange(128)[:, None]
    qt = np.arange(TB)[None, :]
    masks = np.stack(
        [np.where(qt >= kt + 128 * j, 0.0, NEG).astype(np.float32) for j in range(4)],
        axis=1,
    )  # (128, 4, 512)
    masks = np.ascontiguousarray(masks)

    wqk = np.ascontiguousarray(
        (np.asarray(qn_w, np.float32) * np.asarray(kn_w, np.float32)).reshape(128, 1)
    )

    in_maps = []
    for c in range(NCORES):
        qrows = slice(2 * c * D, (2 * c + 2) * D)
        kvrows = slice(c * D, (c + 1) * D)
        wcat = np.concatenate(
            [np.asarray(Wq, np.float32)[qrows], np.asarray(Wk, np.float32)[kvrows],
             np.asarray(Wv, np.float32)[kvrows]], axis=0
        )  # (512, E)
        wqkvT = np.ascontiguousarray(wcat.T)  # (E, 512)
        woT = np.ascontiguousarray(np.asarray(Wo, np.float32)[:, qrows].T)  # (256, E)
        bcat = np.concatenate(
            [np.asarray(bq, np.float32)[qrows], np.asarray(bk, np.float32)[kvrows],
             np.asarray(bv, np.float32)[kvrows]]
        )  # (512,)
        bqkv = np.ascontiguousarray(bcat.reshape(4, 128).T)  # (128, 4)
        in_maps.append(
            {
                "xT": xT,
                "wqkvT": wqkvT,
                "woT": woT,
                "bqkv": bqkv,
                "wqk": wqk,
                "masks": masks,
            }
        )
    return in_maps


def _unshard(results, bo):
    acc = np.zeros((E, BS), np.float64)
    for r in results:
        acc += r["yT"].astype(np.float64)
    y = acc.T.reshape(B, S, E) + np.asarray(bo, np.float64)[None, None, :]
    return y.astype(np.float32)


_NC_CACHE = []


def _get_nc():
    if not _NC_CACHE:
        _NC_CACHE.append(build_nc())
    return _NC_CACHE[0]


def run(inputs, trace=False):
    nc = _get_nc()
    in_maps = _prep_inputs(**inputs)
    res = run_bass_kernel_spmd(
        nc, in_maps, core_ids=list(range(NCORES)), trace=trace
    )
    out = _unshard(res.results, inputs["bo"])
    return out, res


def kernel(**inputs) -> np.ndarray:
    out, _ = run(inputs, trace=False)
    return out
